# revision 3
# baseline (speedup 1.0000x reference)
"""Trainium2 Bass kernel for the CAM (channel attention module) problem.

Computation (per batch b):
    A = inputs[b] reshaped [N=4096, C=512]
    G = A^T A                       (channel Gram matrix, [C, C])
    attn = softmax(G, axis=-1)
    out[b] = gamma * (A @ attn^T) + A

Distribution: pure data-parallel over the batch dim: 16 batches over 8
NeuronCores = 2 batches/core. No collectives.

Design notes (v4):
  - HBM traffic cut from 25.2MB -> 21MB/core: the fp8 gram operand A8 is
    now produced ON-CHIP (DVE/ACT tensor_copy casts from the bf16 A,
    pipelined group-by-group behind the x loads) instead of loading a
    separate host-prepared x8 copy.  A^T stays a host-pretransposed fp8
    load (on-chip transpose would cost more PE/DVE than the 4.2MB DMA).
  - gamma is folded into the softmax weight row (W_rep = gamma/s_i via a
    rank-1 broadcast with a gamma-filled vector instead of ones).  po
    then already contains gamma*(A attn^T), so the residual becomes a
    plain add po + A and can be split across engines:
      k0 tiles: DVE  tensor_tensor(po_psum + A)           (~660ns)
      k1 tiles: ACT  copy po->Sg bf16, DVE add Sg+A       (570 + 424)
      k2/k3:    ACT  copy + (Pool | DVE) add, rotating
    This breaks the v3 bottleneck where DVE alone drained all 64 PSUM
    tiles at 683ns each (43.7us serial on one engine).
  - DMA rings: all loads on sync (HWDGE) in priority order
    x_b0 (first group in 2-nt chunks to start the cast/gram pipeline
    early), x_b1, AT8_b0, AT8_b1.  Stores on gpsimd (SWDGE).
  - Gram accumulates upper-triangle blocks only (G symmetric), fp8 DR.
    Lower-triangle rebuilt by PE transposes; softmax statistic PE ops
    interleave into the next batch's matmul stream (as in v3).
"""

import sys

if "/opt/trn_rl_repo" not in sys.path:
    sys.path.insert(0, "/opt/trn_rl_repo")

import numpy as np

B, H, W, C = 16, 64, 64, 512
N = H * W                 # 4096
NCORES = 8
BPC = B // NCORES         # batches per core = 2
P = 128                   # partitions
NT = N // P               # 32 n-tiles
CT = C // P               # 4 channel tiles
NGRP = 4                  # n-tile groups per batch
GNT = NT // NGRP          # 8 n-tiles per group
OG = 4                    # n-tiles per output store group

_BUILD_CACHE = {}


def _ml_bf16():
    import ml_dtypes

    return np.dtype(ml_dtypes.bfloat16)


def build_bass(gamma_val: float):
    import concourse.bass as bass
    import concourse.bacc as bacc
    import concourse.tile as tile
    from concourse import mybir
    from contextlib import ExitStack

    f32 = mybir.dt.float32
    bf16 = mybir.dt.bfloat16
    f8 = mybir.dt.float8e4
    DR = mybir.MatmulPerfMode.DoubleRow
    Exp = mybir.ActivationFunctionType.Exp
    Alu = mybir.AluOpType
    AX = mybir.AxisListType

    nc = bacc.Bacc("TRN2", target_bir_lowering=False)
    x = nc.dram_tensor("x", [BPC, N, C], bf16, kind="ExternalInput")
    xT8 = nc.dram_tensor("xT8", [BPC, C, N], f8, kind="ExternalInput")
    ident = nc.dram_tensor("ident", [P, P], f32, kind="ExternalInput")
    ones_f = nc.dram_tensor("ones_f", [1, P], f32, kind="ExternalInput")
    gamma_h = nc.dram_tensor("gamma_h", [1, P], bf16, kind="ExternalInput")
    y = nc.dram_tensor("y", [BPC, N, C], bf16, kind="ExternalOutput")

    with tile.TileContext(nc) as tc, ExitStack() as ctx:
        singles = ctx.enter_context(tc.tile_pool(name="singles", bufs=1))
        pA = ctx.enter_context(tc.tile_pool(name="pA", bufs=2))
        pA8 = ctx.enter_context(tc.tile_pool(name="pA8", bufs=2))
        pAT = ctx.enter_context(tc.tile_pool(name="pAT", bufs=2))
        pGs = ctx.enter_context(tc.tile_pool(name="pGs", bufs=2))
        pSm = ctx.enter_context(tc.tile_pool(name="pSm", bufs=2))
        pTmp = ctx.enter_context(tc.tile_pool(name="pTmp", bufs=2))
        pTw = ctx.enter_context(tc.tile_pool(name="pTw", bufs=2))
        pSg = ctx.enter_context(tc.tile_pool(name="pSg", bufs=4))
        pOut = ctx.enter_context(tc.tile_pool(name="pOut", bufs=5))
        pG = ctx.enter_context(tc.tile_pool(name="pG", bufs=4, space="PSUM"))
        pPv = ctx.enter_context(tc.tile_pool(name="pPv", bufs=1, space="PSUM"))
        pPo = ctx.enter_context(tc.tile_pool(name="pPo", bufs=3, space="PSUM"))

        sb_ident = singles.tile([P, P], f32)
        nc.gpsimd.dma_start(out=sb_ident, in_=ident[:, :])
        sb_ones_f = singles.tile([1, P], f32)
        nc.gpsimd.dma_start(out=sb_ones_f, in_=ones_f[:, :])
        sb_gamma_h = singles.tile([1, P], bf16)
        nc.gpsimd.dma_start(out=sb_gamma_h, in_=gamma_h[:, :])

        st = [dict() for _ in range(BPC)]

        def emit_loads(b):
            """bf16 x load.  b0's first group lands in 2-nt chunks so the
            cast -> gram pipeline starts ~3us earlier."""
            Ab = pA.tile([P, NT, C], bf16, name=f"A_b{b}", tag="A")
            st[b]["A"] = Ab
            for g in range(NGRP):
                sl = slice(g * GNT, (g + 1) * GNT)
                src = x[b, g * GNT * P:(g + 1) * GNT * P, :].rearrange(
                    "(nt p) c -> p nt c", p=P
                )
                if b == 0 and g == 0:
                    for h in range(GNT // 2):
                        nc.sync.dma_start(
                            out=Ab[:, 2 * h:2 * h + 2, :],
                            in_=src[:, 2 * h:2 * h + 2, :],
                        )
                else:
                    nc.sync.dma_start(out=Ab[:, sl, :], in_=src)

        # on-chip bf16 -> fp8 cast for the gram operand, per n-group
        def alloc_a8(b):
            st[b]["A8"] = pA8.tile([P, NT, C], f8, name=f"A8_b{b}", tag="A8")

        def emit_cast(b, g, eng, chunks=1):
            Ab, A8b = st[b]["A"], st[b]["A8"]
            n0 = g * GNT
            step = GNT // chunks
            for c0 in range(chunks):
                sl = slice(n0 + c0 * step, n0 + (c0 + 1) * step)
                eng(out=A8b[:, sl, :], in_=Ab[:, sl, :])

        # A^T is a plain load of the host-pretransposed fp8 xT copy.
        def emit_at(b, eng):
            ATb = pAT.tile([P, CT, N], f8, name=f"AT_b{b}", tag="AT")
            eng.dma_start(
                out=ATb,
                in_=xT8[b].rearrange("(jt p) n -> p jt n", p=P),
            )
            st[b]["AT"] = ATb

        # Gram (upper-triangle blocks), with interleaved side ops
        def emit_gram(b, side_ops=()):
            side = list(side_ops)
            A8b = st[b]["A8"]
            G = [
                pG.tile([P, C], f32, name=f"G_b{b}c{ci}", tag="G")
                for ci in range(CT)
            ]
            NP2 = NT // 2
            for t in range(NP2):
                for ci in range(CT):
                    nc.tensor.matmul(
                        G[ci][:, ci * P:],
                        lhsT=A8b[:, 2 * t:2 * t + 2, ci * P:(ci + 1) * P],
                        rhs=A8b[:, 2 * t:2 * t + 2, ci * P:],
                        start=(t == 0),
                        stop=(t == NP2 - 1),
                        perf_mode=DR,
                    )
                if side and t >= 1:
                    side.pop(0)()
            while side:
                side.pop(0)()
            st[b]["G"] = G

        # G rows PSUM->SBUF (frees the G banks) + row-max (negated)
        def emit_stats(b):
            G = st[b]["G"]
            Gs = pGs.tile([P, CT, C], f32, name=f"Gs_b{b}", tag="Gs")
            for ci in range(CT):
                eng = nc.vector.tensor_copy if ci % 2 == 0 else nc.scalar.copy
                eng(out=Gs[:, ci, ci * P:], in_=G[ci][:, ci * P:])
            negm = pSm.tile([P, CT], f32, name=f"negm_b{b}", tag="negm")
            for it in range(CT):
                nc.vector.tensor_reduce(
                    out=negm[:, it:it + 1],
                    in_=Gs[:, it, it * P:],
                    axis=AX.X,
                    op=Alu.max,
                    negate=True,
                )
            st[b]["Gs"] = Gs
            st[b]["negm"] = negm

        # softmax tail as closures, interleaved into the next PE stream.
        def softmax_closures(b, extra_ops=()):
            ve = nc.vector
            cpy = (lambda **kw: nc.vector.tensor_copy(**kw)) if b == 0 else (
                lambda **kw: nc.scalar.copy(**kw))
            Gs = st[b]["Gs"]
            negm = st[b]["negm"]
            s_acc = pSm.tile([P, CT], f32, name=f"s_b{b}", tag="s")
            wrec = pSm.tile([P, CT], f32, name=f"w_b{b}", tag="w")
            Tw8 = pTw.tile([P, CT, C], f8, name=f"Tw8_b{b}", tag="Tw")
            st[b]["Tw"] = Tw8
            extra = list(extra_ops)
            ops = []

            QUADS = [[(1, 0), (2, 0), (2, 1), (3, 0)], [(3, 1), (3, 2)]]
            trq = [None, None]

            def blk_tq(qi, b=b, Gs=Gs):
                trq[qi] = pPv.tile([P, C], f32, name=f"trq_b{b}_{qi}", tag="pv")
                for q, (it, jt) in enumerate(QUADS[qi]):
                    nc.tensor.transpose(
                        out=trq[qi][:, q * P:(q + 1) * P],
                        in_=Gs[:, jt, it * P:(it + 1) * P],
                        identity=sb_ident,
                    )

            def blk_cq(qi, b=b, Gs=Gs):
                for q, (it, jt) in enumerate(QUADS[qi]):
                    cpy(out=Gs[:, it, jt * P:(jt + 1) * P],
                        in_=trq[qi][:, q * P:(q + 1) * P])

            ops.append(lambda: blk_tq(0))
            ops.append(lambda: blk_cq(0))
            ops.append(lambda: blk_tq(1))
            ops.append(lambda: blk_cq(1))

            def s_pass(b=b, Gs=Gs, negm=negm, s_acc=s_acc, wrec=wrec):
                for it in range(CT):
                    S = pTmp.tile([P, C], bf16, name=f"S_b{b}t{it}", tag="S")
                    nc.scalar.activation(
                        out=S,
                        in_=Gs[:, it, :],
                        func=Exp,
                        bias=negm[:, it:it + 1],
                        scale=1.0,
                        accum_out=s_acc[:, it:it + 1],
                    )
                nc.vector.reciprocal(out=wrec, in_=s_acc)

            ops.append(s_pass)
            if extra:
                ops.append(extra.pop(0))

            def col_to_row(src, row):
                vps = pPv.tile([1, C], f32, name=f"vps_{id(row)}", tag="pv")
                for it in range(CT):
                    nc.tensor.transpose(
                        out=vps[0:1, it * P:(it + 1) * P],
                        in_=src[:, it:it + 1],
                        identity=sb_ident,
                    )
                nc.scalar.copy(out=row, in_=vps)

            negm_row = pSm.tile([1, C], f32, name=f"negmrow_b{b}", tag="nrow")
            ops.append(lambda: col_to_row(negm, negm_row))
            w_row = pSm.tile([1, C], bf16, name=f"wrow_b{b}", tag="wrow")
            ops.append(lambda: col_to_row(wrec, w_row))

            NegM_rep = pSm.tile([P, C], f32, name=f"negmrep_b{b}", tag="mrep")
            W_rep = pSm.tile([P, C], bf16, name=f"wrep_b{b}", tag="wrep")

            def rank1(onesv, row, rep):
                ps = pPv.tile([P, C], f32, name=f"rep_{id(rep)}", tag="pv")
                nc.tensor.matmul(ps, lhsT=onesv, rhs=row, start=True, stop=True)
                nc.scalar.copy(out=rep, in_=ps)

            ops.append(lambda: rank1(sb_ones_f, negm_row, NegM_rep))
            # gamma folded here: W_rep[p, i] = gamma * w_i
            ops.append(lambda: rank1(sb_gamma_h, w_row, W_rep))

            # T_w[j, i] = gamma * exp(G[j, i] - m_i) * w_i   (G symmetric)
            def tw_j(jt, b=b, Gs=Gs, Tw8=Tw8):
                tmp = pTmp.tile([P, C], f32, name=f"tmp_b{b}j{jt}", tag="tmp")
                ve.tensor_tensor(
                    out=tmp, in0=Gs[:, jt, :], in1=NegM_rep, op=Alu.add
                )
                Texp = pTmp.tile([P, C], bf16, name=f"Texp_b{b}j{jt}", tag="Texp")
                nc.scalar.activation(out=Texp, in_=tmp, func=Exp)
                ve.tensor_mul(out=Tw8[:, jt, :], in0=Texp, in1=W_rep)

            for jt in range(CT):
                ops.append(lambda jt=jt: tw_j(jt))
                if extra:
                    ops.append(extra.pop(0))
            while extra:
                ops.append(extra.pop(0))
            return ops

        # second matmul + residual (po + A, gamma already in Tw) + store
        def emit_mm2(b, store_eng, side_ops=()):
            side = list(side_ops)
            Ab = st[b]["A"]
            ATb = st[b]["AT"]
            Tw8 = st[b]["Tw"]
            for og in range(NT // OG):
                outg = pOut.tile(
                    [P, OG, C], bf16, name=f"out_b{b}g{og}", tag="out"
                )
                for k in range(OG):
                    nt = og * OG + k
                    po = pPo.tile([P, C], f32, name=f"po_b{b}n{nt}", tag="po")
                    for u in range(CT // 2):
                        nc.tensor.matmul(
                            po,
                            lhsT=ATb[:, 2 * u:2 * u + 2, nt * P:(nt + 1) * P],
                            rhs=Tw8[:, 2 * u:2 * u + 2, :],
                            start=(u == 0),
                            stop=(u == CT // 2 - 1),
                            perf_mode=DR,
                        )
                    # residual: out = po + A, engine-rotated
                    if k == 0 or (k == 3 and og % 3 == 0):
                        nc.vector.tensor_tensor(
                            out=outg[:, k, :], in0=po, in1=Ab[:, nt, :],
                            op=Alu.add,
                        )
                    else:
                        Sgt = pSg.tile([P, C], bf16,
                                       name=f"Sg_b{b}n{nt}", tag="Sg")
                        nc.scalar.copy(out=Sgt, in_=po)
                        use_pool = (k == 2 and og % 2 == 1) or (
                            k == 3 and og % 3 == 2)
                        eng = nc.gpsimd if use_pool else nc.vector
                        eng.tensor_tensor(
                            out=outg[:, k, :], in0=Sgt, in1=Ab[:, nt, :],
                            op=Alu.add,
                        )
                    if side and k % 2 == 1:
                        side.pop(0)()
                store_eng.dma_start(
                    out=y[b, og * OG * P:(og + 1) * OG * P, :].rearrange(
                        "(nt p) c -> p nt c", p=P
                    ),
                    in_=outg,
                )
            while side:
                side.pop(0)()

        # ---- PE warm-up: keep HAM busy before the first loads land -------
        warm_sb = pSm.tile([P, P], bf16, name="warm_sb", tag="warmsb")
        nc.vector.memset(warm_sb, 0.0)
        warm_ps = pPo.tile([P, P], f32, name="warm_ps", tag="po")
        for _ in range(40):
            nc.tensor.matmul(warm_ps, lhsT=warm_sb, rhs=warm_sb,
                             start=True, stop=True)

        # ---- schedule ----------------------------------------------------
        emit_loads(0)                  # sync ring: bf16 x b0 (g0 chunked)
        emit_loads(1)
        emit_at(0, nc.sync)            # fp8 A^T after the x loads
        emit_at(1, nc.sync)
        alloc_a8(0)
        alloc_a8(1)
        # cast b0: first group fine-grained on DVE, then alternate engines
        emit_cast(0, 0, nc.vector.tensor_copy, chunks=4)
        emit_cast(0, 1, nc.scalar.copy)
        emit_cast(0, 2, nc.vector.tensor_copy)
        emit_cast(0, 3, nc.scalar.copy)
        emit_gram(0)
        emit_stats(0)
        # cast b1 early groups (land at ~15/18us, after stats_b0 fires)
        emit_cast(1, 0, nc.scalar.copy)
        emit_cast(1, 1, nc.vector.tensor_copy)
        emit_cast(1, 2, nc.scalar.copy)
        emit_cast(1, 3, nc.vector.tensor_copy)
        emit_gram(1, side_ops=softmax_closures(0))
        emit_stats(1)
        emit_mm2(0, nc.gpsimd, side_ops=softmax_closures(1))
        emit_mm2(1, nc.gpsimd)

    nc.compile()
    return nc


def run(inputs_arr: np.ndarray, gamma_val: float, trace: bool = False):
    """Compile + run on the 8 cores. Returns (output [16,64,64,512], results)."""
    from concourse.bass_utils import run_bass_kernel_spmd

    key = round(float(gamma_val), 12)
    if key not in _BUILD_CACHE:
        _BUILD_CACHE[key] = build_bass(float(gamma_val))
    nc = _BUILD_CACHE[key]

    import ml_dtypes

    bf16 = _ml_bf16()
    f8 = np.dtype(ml_dtypes.float8_e4m3)
    xs = np.asarray(inputs_arr, dtype=np.float32).reshape(B, N, C).astype(bf16)
    xs = np.ascontiguousarray(xs)
    xsT8 = np.ascontiguousarray(xs.astype(f8).transpose(0, 2, 1))
    eye = np.eye(P, dtype=np.float32)
    ones_f = np.ones((1, P), dtype=np.float32)
    gamma_h = np.full((1, P), gamma_val, dtype=np.float32).astype(bf16)
    in_maps = [
        {
            "x": xs[c * BPC:(c + 1) * BPC],
            "xT8": xsT8[c * BPC:(c + 1) * BPC],
            "ident": eye,
            "ones_f": ones_f,
            "gamma_h": gamma_h,
        }
        for c in range(NCORES)
    ]
    res = run_bass_kernel_spmd(nc, in_maps, list(range(NCORES)), trace=trace)
    out = np.concatenate(
        [np.asarray(res.results[c]["y"]) for c in range(NCORES)], axis=0
    )
    return out.astype(np.float32).reshape(B, H, W, C), res


def kernel(inputs: np.ndarray, gamma: np.ndarray) -> np.ndarray:
    gamma_val = float(np.asarray(gamma).reshape(-1)[0])
    out, _ = run(inputs, gamma_val, trace=False)
    return out.astype(np.float32)


if __name__ == "__main__":
    rng = np.random.default_rng(0)
    inp = rng.standard_normal((B, H, W, C), dtype=np.float32)
    gam = np.zeros((1,), dtype=np.float32)
    out = kernel(inp, gam)
    print("shape", out.shape, "dtype", out.dtype)
    print("max|out - inp| =", np.abs(out - inp).max())


# revision 4
# speedup vs baseline: 1.2618x; 1.2618x over previous
"""Trainium2 Bass kernel for the CAM (channel attention module) problem.

Computation (per batch b):
    A = inputs[b] reshaped [N=4096, C=512]
    G = A^T A                       (channel Gram matrix, [C, C])
    attn = softmax(G, axis=-1)
    out[b] = gamma * (A @ attn^T) + A

Distribution: pure data-parallel over the batch dim: 16 batches over 8
NeuronCores = 2 batches/core. No collectives.

Design notes (v5):
  - HBM traffic 21MB/core (vs 25.2 in v3): the fp8 gram operand A8 is
    cast ON-CHIP from the bf16 A (DVE tensor_copy, 4-nt chunks pipelined
    behind the x loads; ACT casts measured 4.5us/group -> DVE only).
  - gamma folded into the softmax weight row (W_rep = gamma/s_i via a
    rank-1 with a gamma-filled vector), so po already holds
    gamma*(A attn^T) and the residual is a plain  po + A.
  - The 64 psum output tiles are drained by a measured-cost balance:
      a-tiles: DVE tensor_tensor(po_psum + A)            (~850ns)
      q-tiles: extra PE matmul  po += I_bf16 @ A_tile    (~320ns PE)
               then a single ACT copy po -> out          (~830ns)
      p-tiles: ACT copy -> Sg, Pool (gpsimd) bf16 add    (~1500ns Pool)
      u-tiles: ACT copy -> Sg, DVE bf16 add              (~500ns DVE)
  - Gs (the symmetrized G copy) is bf16: G ~ 4096 +- 300 and softmax
    has a ~3500 margin to underflow, so bf16 rounding (+-16) is free;
    NegM_rep is bf16 too -> the exp-shift add runs at DVE 2x rate.
    PE transposes of bf16 Gs use a bf16 identity (dtype must match).
  - DMA: loads on sync (x_b0 first group in 2-nt chunks to start the
    cast->gram pipeline early, then x_b1, AT8_b0, AT8_b1).  y_b0 stores
    on sync (after loads), y_b1 on gpsimd; the final store is split
    across both rings to shorten the tail.
"""

import sys

if "/opt/trn_rl_repo" not in sys.path:
    sys.path.insert(0, "/opt/trn_rl_repo")

import numpy as np

B, H, W, C = 16, 64, 64, 512
N = H * W                 # 4096
NCORES = 8
BPC = B // NCORES         # batches per core = 2
P = 128                   # partitions
NT = N // P               # 32 n-tiles
CT = C // P               # 4 channel tiles
NGRP = 4                  # n-tile groups per batch
GNT = NT // NGRP          # 8 n-tiles per group
OG = 4                    # n-tiles per output store group

_BUILD_CACHE = {}


def _ml_bf16():
    import ml_dtypes

    return np.dtype(ml_dtypes.bfloat16)


def build_bass(gamma_val: float):
    import concourse.bass as bass
    import concourse.bacc as bacc
    import concourse.tile as tile
    from concourse import mybir
    from contextlib import ExitStack

    f32 = mybir.dt.float32
    bf16 = mybir.dt.bfloat16
    f8 = mybir.dt.float8e4
    DR = mybir.MatmulPerfMode.DoubleRow
    Exp = mybir.ActivationFunctionType.Exp
    Alu = mybir.AluOpType
    AX = mybir.AxisListType

    nc = bacc.Bacc("TRN2", target_bir_lowering=False)
    x = nc.dram_tensor("x", [BPC, N, C], bf16, kind="ExternalInput")
    xT8 = nc.dram_tensor("xT8", [BPC, C, N], f8, kind="ExternalInput")
    ident = nc.dram_tensor("ident", [P, P], f32, kind="ExternalInput")
    ident_h = nc.dram_tensor("ident_h", [P, P], bf16, kind="ExternalInput")
    ones_f = nc.dram_tensor("ones_f", [1, P], f32, kind="ExternalInput")
    gamma_h = nc.dram_tensor("gamma_h", [1, P], bf16, kind="ExternalInput")
    y = nc.dram_tensor("y", [BPC, N, C], bf16, kind="ExternalOutput")

    with tile.TileContext(nc) as tc, ExitStack() as ctx:
        singles = ctx.enter_context(tc.tile_pool(name="singles", bufs=1))
        pA = ctx.enter_context(tc.tile_pool(name="pA", bufs=2))
        pA8 = ctx.enter_context(tc.tile_pool(name="pA8", bufs=2))
        pAT = ctx.enter_context(tc.tile_pool(name="pAT", bufs=2))
        pGs = ctx.enter_context(tc.tile_pool(name="pGs", bufs=2))
        pSm = ctx.enter_context(tc.tile_pool(name="pSm", bufs=2))
        pTmp = ctx.enter_context(tc.tile_pool(name="pTmp", bufs=2))
        pTw = ctx.enter_context(tc.tile_pool(name="pTw", bufs=2))
        pSg = ctx.enter_context(tc.tile_pool(name="pSg", bufs=3))
        pOut = ctx.enter_context(tc.tile_pool(name="pOut", bufs=5))
        pG = ctx.enter_context(tc.tile_pool(name="pG", bufs=4, space="PSUM"))
        pPv = ctx.enter_context(tc.tile_pool(name="pPv", bufs=1, space="PSUM"))
        pPo = ctx.enter_context(tc.tile_pool(name="pPo", bufs=3, space="PSUM"))

        sb_ident = singles.tile([P, P], f32)
        nc.gpsimd.dma_start(out=sb_ident, in_=ident[:, :])
        sb_ident_h = singles.tile([P, P], bf16)
        nc.gpsimd.dma_start(out=sb_ident_h, in_=ident_h[:, :])
        sb_ones_f = singles.tile([1, P], f32)
        nc.gpsimd.dma_start(out=sb_ones_f, in_=ones_f[:, :])
        sb_gamma_h = singles.tile([1, P], bf16)
        nc.gpsimd.dma_start(out=sb_gamma_h, in_=gamma_h[:, :])

        st = [dict() for _ in range(BPC)]

        def emit_loads(b):
            """bf16 x load.  b0's first group lands in 2-nt chunks so the
            cast -> gram pipeline starts early."""
            Ab = pA.tile([P, NT, C], bf16, name=f"A_b{b}", tag="A")
            st[b]["A"] = Ab
            for g in range(NGRP):
                sl = slice(g * GNT, (g + 1) * GNT)
                src = x[b, g * GNT * P:(g + 1) * GNT * P, :].rearrange(
                    "(nt p) c -> p nt c", p=P
                )
                if b == 0 and g == 0:
                    for h in range(GNT // 2):
                        nc.sync.dma_start(
                            out=Ab[:, 2 * h:2 * h + 2, :],
                            in_=src[:, 2 * h:2 * h + 2, :],
                        )
                else:
                    nc.sync.dma_start(out=Ab[:, sl, :], in_=src)

        # on-chip bf16 -> fp8 cast for the gram operand (DVE only)
        def alloc_a8(b):
            st[b]["A8"] = pA8.tile([P, NT, C], f8, name=f"A8_b{b}", tag="A8")

        def emit_cast(b, fine_first=False):
            Ab, A8b = st[b]["A"], st[b]["A8"]
            chunks = []
            if fine_first:
                chunks += [(i * 2, (i + 1) * 2) for i in range(4)]   # g0 2-nt
                start = GNT
            else:
                start = 0
            n0 = start
            while n0 < NT:
                chunks.append((n0, n0 + 4))
                n0 += 4
            for lo, hi in chunks:
                nc.vector.tensor_copy(
                    out=A8b[:, lo:hi, :], in_=Ab[:, lo:hi, :]
                )

        # A^T is a plain load of the host-pretransposed fp8 xT copy.
        def emit_at(b, eng):
            ATb = pAT.tile([P, CT, N], f8, name=f"AT_b{b}", tag="AT")
            eng.dma_start(
                out=ATb,
                in_=xT8[b].rearrange("(jt p) n -> p jt n", p=P),
            )
            st[b]["AT"] = ATb

        # Gram (upper-triangle blocks), with interleaved side ops
        def emit_gram(b, side_ops=()):
            side = list(side_ops)
            A8b = st[b]["A8"]
            G = [
                pG.tile([P, C], f32, name=f"G_b{b}c{ci}", tag="G")
                for ci in range(CT)
            ]
            NP2 = NT // 2
            for t in range(NP2):
                for ci in range(CT):
                    nc.tensor.matmul(
                        G[ci][:, ci * P:],
                        lhsT=A8b[:, 2 * t:2 * t + 2, ci * P:(ci + 1) * P],
                        rhs=A8b[:, 2 * t:2 * t + 2, ci * P:],
                        start=(t == 0),
                        stop=(t == NP2 - 1),
                        perf_mode=DR,
                    )
                if side and t >= 1:
                    side.pop(0)()
            while side:
                side.pop(0)()
            st[b]["G"] = G

        # G rows PSUM->SBUF as bf16 (frees the G banks) + row-max (negated)
        def emit_stats(b):
            G = st[b]["G"]
            Gs = pGs.tile([P, CT, C], bf16, name=f"Gs_b{b}", tag="Gs")
            for ci in range(CT):
                eng = nc.vector.tensor_copy if ci % 2 == 0 else nc.scalar.copy
                eng(out=Gs[:, ci, ci * P:], in_=G[ci][:, ci * P:])
            negm = pSm.tile([P, CT], f32, name=f"negm_b{b}", tag="negm")
            for it in range(CT):
                nc.vector.tensor_reduce(
                    out=negm[:, it:it + 1],
                    in_=Gs[:, it, it * P:],
                    axis=AX.X,
                    op=Alu.max,
                    negate=True,
                )
            st[b]["Gs"] = Gs
            st[b]["negm"] = negm

        # softmax tail as closures, interleaved into the next PE stream.
        def softmax_closures(b):
            ve = nc.vector
            Gs = st[b]["Gs"]
            negm = st[b]["negm"]
            s_acc = pSm.tile([P, CT], f32, name=f"s_b{b}", tag="s")
            wrec = pSm.tile([P, CT], f32, name=f"w_b{b}", tag="w")
            Tw8 = pTw.tile([P, CT, C], f8, name=f"Tw8_b{b}", tag="Tw")
            st[b]["Tw"] = Tw8
            ops = []

            QUADS = [[(1, 0), (2, 0), (2, 1), (3, 0)], [(3, 1), (3, 2)]]
            trq = [None, None]

            def blk_tq(qi, b=b, Gs=Gs):
                trq[qi] = pPv.tile([P, C], bf16, name=f"trq_b{b}_{qi}",
                                   tag="pv")
                for q, (it, jt) in enumerate(QUADS[qi]):
                    nc.tensor.transpose(
                        out=trq[qi][:, q * P:(q + 1) * P],
                        in_=Gs[:, jt, it * P:(it + 1) * P],
                        identity=sb_ident_h,
                    )

            def blk_cq(qi, b=b, Gs=Gs):
                for q, (it, jt) in enumerate(QUADS[qi]):
                    nc.scalar.copy(out=Gs[:, it, jt * P:(jt + 1) * P],
                                   in_=trq[qi][:, q * P:(q + 1) * P])

            ops.append(lambda: blk_tq(0))
            ops.append(lambda: blk_cq(0))
            ops.append(lambda: blk_tq(1))
            ops.append(lambda: blk_cq(1))

            def s_pass(b=b, Gs=Gs, negm=negm, s_acc=s_acc, wrec=wrec):
                for it in range(CT):
                    S = pTmp.tile([P, C], bf16, name=f"S_b{b}t{it}", tag="S")
                    nc.scalar.activation(
                        out=S,
                        in_=Gs[:, it, :],
                        func=Exp,
                        bias=negm[:, it:it + 1],
                        scale=1.0,
                        accum_out=s_acc[:, it:it + 1],
                    )
                nc.vector.reciprocal(out=wrec, in_=s_acc)

            ops.append(s_pass)

            def col_to_row(src, row):
                vps = pPv.tile([1, C], f32, name=f"vps_{id(row)}", tag="pv")
                for it in range(CT):
                    nc.tensor.transpose(
                        out=vps[0:1, it * P:(it + 1) * P],
                        in_=src[:, it:it + 1],
                        identity=sb_ident,
                    )
                nc.scalar.copy(out=row, in_=vps)

            negm_row = pSm.tile([1, C], f32, name=f"negmrow_b{b}", tag="nrow")
            ops.append(lambda: col_to_row(negm, negm_row))
            w_row = pSm.tile([1, C], bf16, name=f"wrow_b{b}", tag="wrow")
            ops.append(lambda: col_to_row(wrec, w_row))

            NegM_rep = pSm.tile([P, C], bf16, name=f"negmrep_b{b}", tag="mrep")
            W_rep = pSm.tile([P, C], bf16, name=f"wrep_b{b}", tag="wrep")

            def rank1(onesv, row, rep):
                ps = pPv.tile([P, C], f32, name=f"rep_{id(rep)}", tag="pv")
                nc.tensor.matmul(ps, lhsT=onesv, rhs=row, start=True, stop=True)
                nc.scalar.copy(out=rep, in_=ps)

            ops.append(lambda: rank1(sb_ones_f, negm_row, NegM_rep))
            # gamma folded here: W_rep[p, i] = gamma * w_i
            ops.append(lambda: rank1(sb_gamma_h, w_row, W_rep))

            # T_w[j, i] = gamma * exp(G[j, i] - m_i) * w_i   (G symmetric)
            def tw_j(jt, b=b, Gs=Gs, Tw8=Tw8):
                tmp = pTmp.tile([P, C], bf16, name=f"tmp_b{b}j{jt}", tag="tmp")
                ve.tensor_tensor(
                    out=tmp, in0=Gs[:, jt, :], in1=NegM_rep, op=Alu.add
                )
                Texp = pTmp.tile([P, C], bf16, name=f"Texp_b{b}j{jt}",
                                 tag="Texp")
                nc.scalar.activation(out=Texp, in_=tmp, func=Exp)
                ve.tensor_mul(out=Tw8[:, jt, :], in0=Texp, in1=W_rep)

            for jt in range(CT):
                ops.append(lambda jt=jt: tw_j(jt))
            return ops

        # second matmul + residual (po + A, gamma already in Tw) + store
        def emit_mm2(b, side_ops=()):
            side = list(side_ops)
            Ab = st[b]["A"]
            ATb = st[b]["AT"]
            Tw8 = st[b]["Tw"]
            NOG = NT // OG
            for og in range(NOG):
                outg = pOut.tile(
                    [P, OG, C], bf16, name=f"out_b{b}g{og}", tag="out"
                )
                for k in range(OG):
                    nt = og * OG + k
                    # tile kind: a=DVE-TT, q=PE-ident+ACT, p=ACT+Pool,
                    # u=ACT+DVE-add
                    if k == 0 or k == 2:
                        kind = "a"
                    elif k == 1:
                        kind = "q"
                    else:
                        kind = ("p", "q", "a", "q", "p", "u", "p", "u")[og]
                    po = pPo.tile([P, C], f32, name=f"po_b{b}n{nt}", tag="po")
                    for u in range(CT // 2):
                        nc.tensor.matmul(
                            po,
                            lhsT=ATb[:, 2 * u:2 * u + 2, nt * P:(nt + 1) * P],
                            rhs=Tw8[:, 2 * u:2 * u + 2, :],
                            start=(u == 0),
                            stop=(u == CT // 2 - 1 and kind != "q"),
                            perf_mode=DR,
                        )
                    if kind == "q":
                        # po += I @ A_tile on the PE; drain is then 1 ACT copy
                        nc.tensor.matmul(
                            po,
                            lhsT=sb_ident_h,
                            rhs=Ab[:, nt, :],
                            start=False,
                            stop=True,
                        )
                        nc.scalar.copy(out=outg[:, k, :], in_=po)
                    elif kind == "a":
                        nc.vector.tensor_tensor(
                            out=outg[:, k, :], in0=po, in1=Ab[:, nt, :],
                            op=Alu.add,
                        )
                    else:
                        Sgt = pSg.tile([P, C], bf16,
                                       name=f"Sg_b{b}n{nt}", tag="Sg")
                        nc.scalar.copy(out=Sgt, in_=po)
                        eng = nc.gpsimd if kind == "p" else nc.vector
                        eng.tensor_tensor(
                            out=outg[:, k, :], in0=Sgt, in1=Ab[:, nt, :],
                            op=Alu.add,
                        )
                    if side and k % 2 == 1:
                        side.pop(0)()
                dst = y[b, og * OG * P:(og + 1) * OG * P, :].rearrange(
                    "(nt p) c -> p nt c", p=P
                )
                if b == 1 and og == NOG - 1:
                    # split the final store across both rings (shorter tail)
                    half = OG // 2
                    nc.gpsimd.dma_start(out=dst[:, :half, :],
                                        in_=outg[:, :half, :])
                    nc.sync.dma_start(out=dst[:, half:, :],
                                      in_=outg[:, half:, :])
                else:
                    eng = nc.sync if b == 0 else nc.gpsimd
                    eng.dma_start(out=dst, in_=outg)
            while side:
                side.pop(0)()

        # ---- PE warm-up: keep HAM busy before the first loads land -------
        warm_sb = pSm.tile([P, P], bf16, name="warm_sb", tag="warmsb")
        nc.vector.memset(warm_sb, 0.0)
        warm_ps = pPo.tile([P, P], f32, name="warm_ps", tag="po")
        for _ in range(40):
            nc.tensor.matmul(warm_ps, lhsT=warm_sb, rhs=warm_sb,
                             start=True, stop=True)

        # ---- schedule ----------------------------------------------------
        emit_loads(0)                  # sync ring: bf16 x b0 (g0 chunked)
        emit_loads(1)
        emit_at(0, nc.sync)            # fp8 A^T after the x loads
        emit_at(1, nc.sync)
        alloc_a8(0)
        alloc_a8(1)
        emit_cast(0, fine_first=True)
        emit_gram(0)
        emit_stats(0)
        emit_cast(1)
        emit_gram(1, side_ops=softmax_closures(0))
        emit_stats(1)
        emit_mm2(0, side_ops=softmax_closures(1))
        emit_mm2(1)

    nc.compile()
    return nc


def run(inputs_arr: np.ndarray, gamma_val: float, trace: bool = False):
    """Compile + run on the 8 cores. Returns (output [16,64,64,512], results)."""
    from concourse.bass_utils import run_bass_kernel_spmd

    key = round(float(gamma_val), 12)
    if key not in _BUILD_CACHE:
        _BUILD_CACHE[key] = build_bass(float(gamma_val))
    nc = _BUILD_CACHE[key]

    import ml_dtypes

    bf16 = _ml_bf16()
    f8 = np.dtype(ml_dtypes.float8_e4m3)
    xs = np.asarray(inputs_arr, dtype=np.float32).reshape(B, N, C).astype(bf16)
    xs = np.ascontiguousarray(xs)
    xsT8 = np.ascontiguousarray(xs.astype(f8).transpose(0, 2, 1))
    eye = np.eye(P, dtype=np.float32)
    eye_h = eye.astype(bf16)
    ones_f = np.ones((1, P), dtype=np.float32)
    gamma_h = np.full((1, P), gamma_val, dtype=np.float32).astype(bf16)
    in_maps = [
        {
            "x": xs[c * BPC:(c + 1) * BPC],
            "xT8": xsT8[c * BPC:(c + 1) * BPC],
            "ident": eye,
            "ident_h": eye_h,
            "ones_f": ones_f,
            "gamma_h": gamma_h,
        }
        for c in range(NCORES)
    ]
    res = run_bass_kernel_spmd(nc, in_maps, list(range(NCORES)), trace=trace)
    out = np.concatenate(
        [np.asarray(res.results[c]["y"]) for c in range(NCORES)], axis=0
    )
    return out.astype(np.float32).reshape(B, H, W, C), res


def kernel(inputs: np.ndarray, gamma: np.ndarray) -> np.ndarray:
    gamma_val = float(np.asarray(gamma).reshape(-1)[0])
    out, _ = run(inputs, gamma_val, trace=False)
    return out.astype(np.float32)


if __name__ == "__main__":
    rng = np.random.default_rng(0)
    inp = rng.standard_normal((B, H, W, C), dtype=np.float32)
    gam = np.zeros((1,), dtype=np.float32)
    out = kernel(inp, gam)
    print("shape", out.shape, "dtype", out.dtype)
    print("max|out - inp| =", np.abs(out - inp).max())
